# revision 12
# baseline (speedup 1.0000x reference)
"""Trainium2 Bass kernel for nn_CustomRGCN (2-layer RGCN + BN + MLP).

Strategy (8 NeuronCores, SPMD, one NEFF), v2:
- Nodes sharded by contiguous range: core c owns nodes [c*6250, (c+1)*6250),
  padded to 6272 = 49*128 rows per shard. All activations fp16 (psum f32).
- Dense transforms channel-major ([ch, nodes]), node tiles of 512, weights
  fp16 resident in SBUF; activations resident in SBUF (no DRAM spills).
- RGCN aggregation: per 128-edge tile, an indirect DMA gathers the edge-source
  rows (fp16, from the AllGather'ed node-major table; one offset per partition
  - multi-offset batching is NOT supported by the HW SWDGE); segment-mean via
  matmul with host-built one-hot S pre-scaled by 1/cnt (fp16), with the
  gathered tile as the stationary operand so means come out channel-major
  directly (no PE transpose / no per-tile rescale).
- Root-path matmuls issued before the gather-dependent work so they overlap
  the AllGather; combined with the relation part via a fused DVE add that
  also emits the BN sum; sumsq via Activation Square with accum_out.
- b_rgcn dropped (training-mode BN cancels constant channel shifts).
- BN stats AllReduce'd ([2,320] f32); y = Lrelu(h*s+t) fused on Act engine.
- Collectives: AllGather of each layer's node-major fp16 shard into a
  Shared-scratchpad table; AllReduce of stats. Edge routing/sorting/padding
  and S construction happen on host in numpy.
"""
import sys
sys.path.insert(0, '/opt/trn_rl_repo')

import numpy as np

N = 50000
E = 500000
D = 320
R = 3
NCORES = 8
NC_NODES = N // NCORES          # 6250
P = 128
NT = (NC_NODES + P - 1) // P    # 49 node tiles per core
NPAD = NT * P                   # 6272
NBLK = NT * R                   # 147 blocks per core
NG = 512
TPG = NG // P                   # 4 dst tiles per group
EPS = 1e-5
LRELU_ALPHA = 0.01

# groups of node columns: (g0, ng, real, t_lo)
_GROUPS = []
_g0 = 0
while _g0 < NPAD:
    _ng = min(NG, NPAD - _g0)
    _real = max(0, min(NC_NODES - _g0, _ng))
    _GROUPS.append((_g0, _ng, _real, _g0 // P))
    _g0 += _ng

KCH = [(0, 128), (128, 128), (256, 64)]
KIN = [(0, 128), (128, 64), (192, 64), (256, 64)]


def _prep_graph(edge_index, edge_type):
    """Route edges to dst shards, sort by (node-tile, rel) block, build
    per-tile gather indices and the 1/cnt-scaled one-hot S (transposed)."""
    src = np.asarray(edge_index[0], dtype=np.int64)
    dst = np.asarray(edge_index[1], dtype=np.int64)
    rel = np.asarray(edge_type, dtype=np.int64)

    owner = dst // NC_NODES
    dstl = dst - owner * NC_NODES
    src_ag = (src // NC_NODES) * NPAD + (src % NC_NODES)
    bid = (dstl // P) * R + rel
    slot = dstl % P
    seg = dstl * R + rel

    cnts = np.zeros((NCORES, NBLK), np.int64)
    for c in range(NCORES):
        cnts[c] = np.bincount(bid[owner == c], minlength=NBLK)
    tiles_pb = np.maximum(1, -(-cnts.max(axis=0) // P))
    tile_start = np.zeros(NBLK + 1, np.int64)
    np.cumsum(tiles_pb, out=tile_start[1:])
    T = int(tile_start[-1])

    tsrc = np.zeros((NCORES, T * P), np.int32)
    S_T = np.zeros((NCORES, P, T * P), np.float16)  # [P(edge), T*P(tile,slot)]

    for c in range(NCORES):
        m = owner == c
        bid_c, slot_c, src_c, seg_c = bid[m], slot[m], src_ag[m], seg[m]
        segcnt = np.bincount(seg_c, minlength=NC_NODES * R)
        icnt_e = (1.0 / segcnt[seg_c]).astype(np.float32)
        order = np.argsort(bid_c, kind='stable')
        bid_s = bid_c[order]
        cnt_c = cnts[c]
        within = np.arange(len(bid_s)) - np.repeat(
            np.concatenate(([0], np.cumsum(cnt_c)[:-1])), cnt_c)
        pos = tile_start[:-1][bid_s] * P + within
        tsrc[c, pos] = src_c[order]
        e_in_tile = pos % P
        t_of = pos // P
        S_T[c, e_in_tile, t_of * P + slot_c[order]] = icnt_e[order]

    tsrc_cols = tsrc.reshape(NCORES, T, P).transpose(0, 2, 1).copy()
    return dict(T=T, tile_start=tile_start, S_T=S_T, tsrc_cols=tsrc_cols)


def _shard_T(x, dtype=np.float16):
    F = x.shape[1]
    out = np.zeros((NCORES, F, NPAD), dtype)
    for c in range(NCORES):
        out[c, :, :NC_NODES] = np.asarray(x[c * NC_NODES:(c + 1) * NC_NODES].T,
                                          dtype=dtype)
    return out


def _build_nc(T, tile_start, profile=False):
    import concourse.bass as bass
    import concourse.tile as tile
    import concourse.bacc as bacc
    import concourse.mybir as mybir
    from concourse.masks import make_identity
    from contextlib import ExitStack

    f32 = mybir.dt.float32
    f16 = mybir.dt.float16
    i32 = mybir.dt.int32

    # max tiles in one dst-tile's 3-relation batch
    ntmax = max(int(tile_start[(t + 1) * R] - tile_start[t * R])
                for t in range(NT))

    nc = bacc.Bacc("TRN2", target_bir_lowering=False, debug=False,
                   num_devices=1 if profile else NCORES)

    def din(name, shape, dt=f16):
        return nc.dram_tensor(name, shape, dt, kind="ExternalInput")

    ncat_T = din("ncat_T", [8, NPAD])
    tweet_T = din("tweet_T", [768, NPAD])
    user_T = din("user_T", [768, NPAD])
    uname_T = din("uname_T", [768, NPAD])
    w_ncat = din("w_ncat", [8, 128])
    b_c0 = din("b_c0", [128, 1], f32)
    w_tw = din("w_tw", [768, 64])
    w_us = din("w_us", [768, 64])
    w_un = din("w_un", [768, 64])
    b_tw = din("b_tw", [64, 1], f32)
    b_us = din("b_us", [64, 1], f32)
    b_c2 = din("b_c2", [64, 1], f32)
    w_in = din("w_in", [D, D])
    b_in = din("b_in", [D, 1], f32)
    w_rel = [din(f"w_rel{l}", [R, D, D]) for l in (1, 2)]
    w_root = [din(f"w_root{l}", [D, D]) for l in (1, 2)]
    gamma = [din(f"gamma{l}", [D, 1], f32) for l in (1, 2)]
    beta = [din(f"beta{l}", [D, 1], f32) for l in (1, 2)]
    w_o1 = din("w_o1", [D, D])
    b_o1 = din("b_o1", [D, 1], f32)
    w_o2 = din("w_o2", [D, D])
    b_o2 = din("b_o2", [D, 1], f32)
    w_o3 = din("w_o3", [D, 2])
    b_o3 = din("b_o3", [2, 1], f32)
    s_matT = din("s_matT", [P, T * P])
    tsrc = nc.dram_tensor("tsrc", [P, T], i32, kind="ExternalInput")

    out = nc.dram_tensor("out", [NC_NODES, 2], f32, kind="ExternalOutput")

    agi = [nc.dram_tensor(f"agi{l}", [NPAD, D], f16) for l in (1, 2)]
    ago_kw = {} if profile else dict(addr_space="Shared")
    ago = [nc.dram_tensor(f"ago{l}", [NCORES * NPAD, D], f16, **ago_kw)
           for l in (1, 2)]
    arin = [nc.dram_tensor(f"arin{l}", [2, D], f32) for l in (1, 2)]
    arout = [nc.dram_tensor(f"arout{l}", [2, D], f32) for l in (1, 2)]

    RG = [list(range(NCORES))]

    with tile.TileContext(nc) as tc:
        ctx = ExitStack()
        wp = ctx.enter_context(tc.tile_pool(name="weights", bufs=1))
        cp = ctx.enter_context(tc.tile_pool(name="consts", bufs=1))
        xs = ctx.enter_context(tc.tile_pool(name="xstore", bufs=1))
        hs = ctx.enter_context(tc.tile_pool(name="hstore", bufs=1))
        sbg = ctx.enter_context(tc.tile_pool(name="gather", bufs=1))
        sba = ctx.enter_context(tc.tile_pool(name="acts", bufs=1))
        sbt = ctx.enter_context(tc.tile_pool(name="stats", bufs=1))
        psB = ctx.enter_context(tc.tile_pool(name="psB", bufs=2, space="PSUM"))
        psM = ctx.enter_context(tc.tile_pool(name="psM", bufs=2, space="PSUM"))
        psT = ctx.enter_context(tc.tile_pool(name="psT", bufs=2, space="PSUM"))

        ident = cp.tile([P, P], f16)
        make_identity(nc, ident[:])
        eps_sb = cp.tile([P, 1], f32)
        nc.gpsimd.memset(eps_sb[:], EPS)
        idxt = cp.tile([P, T], i32)
        nc.sync.dma_start(out=idxt[:], in_=tsrc[:, :])

        def wload(name, ap, kk, cols):
            t = wp.tile([kk, cols], f16, name=name)
            nc.sync.dma_start(out=t[:], in_=ap)
            return t

        w_nc_sb = wload("w_nc_sb", w_ncat[:, :], 8, 128)
        w_tw_sb = [wload(f"w_tw{j}", w_tw[j*128:(j+1)*128, :], 128, 64) for j in range(6)]
        w_us_sb = [wload(f"w_us{j}", w_us[j*128:(j+1)*128, :], 128, 64) for j in range(6)]
        w_un_sb = [wload(f"w_un{j}", w_un[j*128:(j+1)*128, :], 128, 64) for j in range(6)]
        w_in_sb = [wload(f"w_in{k}", w_in[k0:k0+kk, :], kk, D) for k, (k0, kk) in enumerate(KIN)]
        w_rel_sb = [[[wload(f"w_rel{l}_{r}_{k}", w_rel[l][r, k0:k0+kk, :], kk, D)
                      for k, (k0, kk) in enumerate(KCH)] for r in range(R)]
                    for l in (0, 1)]
        w_root_sb = [[wload(f"w_root{l}_{k}", w_root[l][k0:k0+kk, :], kk, D)
                      for k, (k0, kk) in enumerate(KCH)] for l in (0, 1)]
        w_o1_sb = [wload(f"w_o1_{k}", w_o1[k0:k0+kk, :], kk, D) for k, (k0, kk) in enumerate(KCH)]
        w_o2_sb = [wload(f"w_o2_{k}", w_o2[k0:k0+kk, :], kk, D) for k, (k0, kk) in enumerate(KCH)]
        w_o3_sb = [wload(f"w_o3_{k}", w_o3[k0:k0+kk, :], kk, 2) for k, (k0, kk) in enumerate(KCH)]

        def bload(name, ap, kk):
            t = cp.tile([kk, 1], f32, name=name)
            nc.sync.dma_start(out=t[:], in_=ap)
            return t

        b_c0_sb = bload("b_c0_sb", b_c0[:, :], 128)
        b_tw_sb = bload("b_tw_sb", b_tw[:, :], 64)
        b_us_sb = bload("b_us_sb", b_us[:, :], 64)
        b_c2_sb = bload("b_c2_sb", b_c2[:, :], 64)
        b_in_sb = [bload(f"b_in{k}", b_in[k0:k0+kk, :], kk) for k, (k0, kk) in enumerate(KCH)]
        b_o1_sb = [bload(f"b_o1_{k}", b_o1[k0:k0+kk, :], kk) for k, (k0, kk) in enumerate(KCH)]
        b_o2_sb = [bload(f"b_o2_{k}", b_o2[k0:k0+kk, :], kk) for k, (k0, kk) in enumerate(KCH)]
        b_o3_sb = bload("b_o3_sb", b_o3[:, :], 2)
        gamma_sb = [[bload(f"gam{l}_{k}", gamma[l][k0:k0+kk, :], kk)
                     for k, (k0, kk) in enumerate(KCH)] for l in (0, 1)]
        beta_sb = [[bload(f"bet{l}_{k}", beta[l][k0:k0+kk, :], kk)
                    for k, (k0, kk) in enumerate(KCH)] for l in (0, 1)]

        Lrelu = mybir.ActivationFunctionType.Lrelu
        Square = mybir.ActivationFunctionType.Square
        Sqrt = mybir.ActivationFunctionType.Sqrt
        Ident = mybir.ActivationFunctionType.Identity
        AX = mybir.AxisListType.X
        ALU = mybir.AluOpType

        # SBUF-resident per-group activation tiles (overwritten layer by layer)
        xa = [[xs.tile([mm, NG], f16, name=f"xa_{gi}_{m}", tag=f"xa_{gi}_{m}")
               for m, (m0, mm) in enumerate(KCH)]
              for gi in range(len(_GROUPS))]
        hsb = [[hs.tile([mm, NG], f16, name=f"h_{gi}_{m}", tag=f"h_{gi}_{m}")
                for m, (m0, mm) in enumerate(KCH)]
               for gi in range(len(_GROUPS))]

        # engine round-robin for psum->sbuf copies
        _cp_state = [0]

        def copy_out(dst_ap, src_ap):
            if _cp_state[0] % 2 == 0:
                nc.vector.tensor_copy(dst_ap, src_ap)
            else:
                nc.scalar.activation(dst_ap, src_ap, Ident)
            _cp_state[0] += 1

        def nm_emit(l, gi, g0, ng, src_tiles):
            """Transpose cm fp16 group tiles -> node-major rows of agi[l]."""
            nj = ng // P
            tnm = sba.tile([P, TPG * D], f16, tag="tnm", bufs=2)
            for j in range(nj):
                for m, (m0, mm) in enumerate(KCH):
                    pt = psT.tile([P, P], f16, tag="pt", bufs=2)
                    nc.tensor.transpose(out=pt[:, :],
                                        in_=src_tiles[m][:mm, j*P:(j+1)*P],
                                        identity=ident[:mm, :])
                    copy_out(tnm[:, j*D+m0:j*D+m0+mm], pt[:, :mm])
            ag3 = agi[l].ap().rearrange("(j p) c -> p j c", p=P)
            t_lo = g0 // P
            nc.sync.dma_start(out=ag3[:, t_lo:t_lo+nj, :],
                              in_=tnm[:, :nj*D])

        # =========================================================
        # Phase 1: input projection -> x (cm fp16 in SBUF) + agi[0]
        # =========================================================
        emb3 = [tab.ap().rearrange("(j p) n -> p j n", p=P)
                for tab in (tweet_T, user_T, uname_T)]
        for gi, (g0, ng, real, t_lo) in enumerate(_GROUPS):
            x0 = []
            nct = sbg.tile([8, NG], f16, tag="nct", bufs=2)
            nc.sync.dma_start(out=nct[:, :ng], in_=ncat_T[:, g0:g0+ng])
            pa = psB.tile([P, NG], f32, tag="pb")
            nc.tensor.matmul(pa[:, :ng], w_nc_sb[:], nct[:, :ng],
                             start=True, stop=True)
            t00 = sba.tile([P, NG], f16, tag="x00", bufs=2)
            nc.scalar.activation(t00[:, :ng], pa[:, :ng], Lrelu,
                                 bias=b_c0_sb[:, :1], alpha=LRELU_ALPHA)
            x0.append(t00)
            for piece, (wsb, bsb) in enumerate(
                    [(w_tw_sb, b_tw_sb), (w_us_sb, b_us_sb), (w_un_sb, b_c2_sb)]):
                embt = sbg.tile([P, 6 * NG], f16, name="embt", tag="emb", bufs=2)
                nc.sync.dma_start(out=embt[:, :6*ng],
                                  in_=emb3[piece][:, :, g0:g0+ng])
                pa = psB.tile([P, NG], f32, tag="pb")
                for j in range(6):
                    nc.tensor.matmul(pa[0:64, :ng], wsb[j][:],
                                     embt[:, j*ng:(j+1)*ng],
                                     start=(j == 0), stop=(j == 5))
                tp = sba.tile([64, NG], f16, tag=f"x0{piece+1}", bufs=2)
                nc.scalar.activation(tp[0:64, :ng], pa[0:64, :ng], Lrelu,
                                     bias=bsb[:, :1], alpha=LRELU_ALPHA)
                x0.append(tp)
            for m, (m0, mm) in enumerate(KCH):
                pb = psB.tile([P, NG], f32, tag="pb")
                for k, (k0, kk) in enumerate(KIN):
                    nc.tensor.matmul(pb[:mm, :ng], w_in_sb[k][:kk, m0:m0+mm],
                                     x0[k][:kk, :ng], start=(k == 0), stop=(k == 3))
                nc.scalar.activation(xa[gi][m][:mm, :ng], pb[:mm, :ng], Lrelu,
                                     bias=b_in_sb[m][:, :1], alpha=LRELU_ALPHA)
            nm_emit(0, gi, g0, ng, xa[gi])

        if profile:
            nc.sync.dma_start(out=ago[0][0:NPAD, :], in_=agi[0][:, :])
        else:
            nc.gpsimd.collective_compute(
                "AllGather", mybir.AluOpType.bypass, replica_groups=RG,
                ins=[agi[0].ap().opt()], outs=[ago[0].ap().opt()])

        # =========================================================
        # RGCN layers
        # =========================================================
        for l in range(2):
            src_tab = ago[l]
            sum_st = [sbt.tile([mm, 16], f32, name=f"sum{l}_{m}")
                      for m, (m0, mm) in enumerate(KCH)]
            sq_st = [sbt.tile([mm, 16], f32, name=f"sq{l}_{m}")
                     for m, (m0, mm) in enumerate(KCH)]
            for m in range(3):
                nc.vector.memset(sum_st[m][:], 0.0)
                nc.vector.memset(sq_st[m][:], 0.0)

            # ---- root matmuls first: overlap the AllGather ----
            for gi, (g0, ng, real, t_lo) in enumerate(_GROUPS):
                for m, (m0, mm) in enumerate(KCH):
                    pb = psB.tile([P, NG], f32, tag="pb")
                    for k, (k0, kk) in enumerate(KCH):
                        nc.tensor.matmul(pb[:mm, :ng],
                                         w_root_sb[l][k][:kk, m0:m0+mm],
                                         xa[gi][k][:kk, :ng],
                                         start=(k == 0), stop=(k == 2))
                    nc.scalar.activation(hsb[gi][m][:mm, :ng], pb[:mm, :ng],
                                         Ident)

            # ---- aggregation: gather + segment-mean + relation transform ----
            mcm = None
            for gi, (g0, ng, real, t_lo) in enumerate(_GROUPS):
                mcm = [[sba.tile([kk, NG], f16, name=f"mcm{r}_{k}", tag=f"mcm{r}_{k}", bufs=2)
                        for k, (k0, kk) in enumerate(KCH)] for r in range(R)]
                for t in range(t_lo, t_lo + ng // P):
                    i0 = int(tile_start[t * R])
                    i1 = int(tile_start[(t + 1) * R])
                    nt = i1 - i0
                    slot4 = t - t_lo
                    gb = sbg.tile([P, ntmax * D], f16, tag="gb", bufs=2)
                    for j in range(nt):
                        nc.gpsimd.indirect_dma_start(
                            out=gb[:, j*D:(j+1)*D], out_offset=None,
                            in_=src_tab[:, :],
                            in_offset=bass.IndirectOffsetOnAxis(
                                ap=idxt[:, i0+j:i0+j+1], axis=0))
                    st = sbg.tile([P, ntmax * P], f16, tag="st", bufs=2)
                    nc.sync.dma_start(out=st[:, :nt*P],
                                      in_=s_matT[:, i0*P:i1*P])
                    for r in range(R):
                        j0 = int(tile_start[t * R + r]) - i0
                        j1 = int(tile_start[t * R + r + 1]) - i0
                        pm = psM.tile([P, 3 * P], f32, tag="pm", bufs=4)
                        for m, (m0, mm) in enumerate(KCH):
                            for j in range(j0, j1):
                                nc.tensor.matmul(
                                    pm[:mm, m*P:(m+1)*P],
                                    gb[:, j*D+m0:j*D+m0+mm],
                                    st[:, j*P:(j+1)*P],
                                    start=(j == j0), stop=(j == j1 - 1))
                        for m, (m0, mm) in enumerate(KCH):
                            copy_out(mcm[r][m][:mm, slot4*P:(slot4+1)*P],
                                     pm[:mm, m*P:(m+1)*P])
                # relation transform + combine with root + stats
                for m, (m0, mm) in enumerate(KCH):
                    pb = psB.tile([P, NG], f32, tag="pb")
                    first = True
                    for r in range(R):
                        for k, (k0, kk) in enumerate(KCH):
                            nc.tensor.matmul(pb[:mm, :ng],
                                             w_rel_sb[l][r][k][:kk, m0:m0+mm],
                                             mcm[r][k][:kk, :ng],
                                             start=first,
                                             stop=(r == 2 and k == 2))
                            first = False
                    # h = pb + h_root (fused add + BN sum accumulation)
                    nc.vector.scalar_tensor_tensor(
                        out=hsb[gi][m][:mm, :real],
                        in0=pb[:mm, :real], scalar=1.0,
                        in1=hsb[gi][m][:mm, :real],
                        op0=ALU.mult, op1=ALU.add,
                        accum_out=sum_st[m][:mm, gi:gi+1])
                    if real < ng:
                        nc.vector.scalar_tensor_tensor(
                            out=hsb[gi][m][:mm, real:ng],
                            in0=pb[:mm, real:ng], scalar=1.0,
                            in1=hsb[gi][m][:mm, real:ng],
                            op0=ALU.mult, op1=ALU.add)
                    sqs = sba.tile([P, NG], f16, name="sqs", tag="sq", bufs=2)
                    nc.scalar.activation(sqs[:mm, :real],
                                         hsb[gi][m][:mm, :real], Square,
                                         accum_out=sq_st[m][:mm, gi:gi+1])

            # ---- BN stats AllReduce ----
            for m, (m0, mm) in enumerate(KCH):
                s1 = sbt.tile([mm, 1], f32, name=f"s1_{l}_{m}")
                s2 = sbt.tile([mm, 1], f32, name=f"s2_{l}_{m}")
                nc.vector.reduce_sum(s1[:mm, :], sum_st[m][:mm, :], axis=AX)
                nc.vector.reduce_sum(s2[:mm, :], sq_st[m][:mm, :], axis=AX)
                nc.sync.dma_start(out=arin[l][0:1, m0:m0+mm], in_=s1[:mm, :])
                nc.sync.dma_start(out=arin[l][1:2, m0:m0+mm], in_=s2[:mm, :])
            if profile:
                nc.sync.dma_start(out=arout[l][:, :], in_=arin[l][:, :])
            else:
                nc.gpsimd.collective_compute(
                    "AllReduce", mybir.AluOpType.add, replica_groups=RG,
                    ins=[arin[l].ap().opt()], outs=[arout[l].ap().opt()])

            scl, sft = [], []
            for m, (m0, mm) in enumerate(KCH):
                sg = sbt.tile([mm, 1], f32, name=f"sg_{l}_{m}")
                sqg = sbt.tile([mm, 1], f32, name=f"sqg_{l}_{m}")
                nc.sync.dma_start(out=sg[:mm, :], in_=arout[l][0:1, m0:m0+mm])
                nc.sync.dma_start(out=sqg[:mm, :], in_=arout[l][1:2, m0:m0+mm])
                mean = sbt.tile([mm, 1], f32, name=f"mean_{l}_{m}")
                nc.vector.tensor_scalar_mul(mean[:mm, :], sg[:mm, :], 1.0 / N)
                msq = sbt.tile([mm, 1], f32, name=f"msq_{l}_{m}")
                nc.vector.tensor_scalar_mul(msq[:mm, :], sqg[:mm, :], 1.0 / N)
                m2 = sbt.tile([mm, 1], f32, name=f"m2_{l}_{m}")
                nc.vector.tensor_tensor(out=m2[:mm, :], in0=mean[:mm, :],
                                        in1=mean[:mm, :], op=ALU.mult)
                var = sbt.tile([mm, 1], f32, name=f"var_{l}_{m}")
                nc.vector.tensor_tensor(out=var[:mm, :], in0=msq[:mm, :],
                                        in1=m2[:mm, :], op=ALU.subtract)
                nc.vector.tensor_tensor(out=var[:mm, :], in0=var[:mm, :],
                                        in1=eps_sb[:mm, :], op=ALU.add)
                std = sbt.tile([mm, 1], f32, name=f"std_{l}_{m}")
                nc.scalar.activation(std[:mm, :], var[:mm, :], Sqrt)
                istd = sbt.tile([mm, 1], f32, name=f"istd_{l}_{m}")
                nc.vector.reciprocal(istd[:mm, :], std[:mm, :])
                sc = sbt.tile([mm, 1], f32, name=f"sc_{l}_{m}")
                nc.vector.tensor_tensor(out=sc[:mm, :], in0=gamma_sb[l][m][:mm, :],
                                        in1=istd[:mm, :], op=ALU.mult)
                tmp = sbt.tile([mm, 1], f32, name=f"tmp_{l}_{m}")
                nc.vector.tensor_tensor(out=tmp[:mm, :], in0=mean[:mm, :],
                                        in1=sc[:mm, :], op=ALU.mult)
                sh = sbt.tile([mm, 1], f32, name=f"sh_{l}_{m}")
                nc.vector.tensor_tensor(out=sh[:mm, :], in0=beta_sb[l][m][:mm, :],
                                        in1=tmp[:mm, :], op=ALU.subtract)
                scl.append(sc)
                sft.append(sh)

            # ---- normalize + lrelu -> y (overwrites xa in place) ----
            for gi, (g0, ng, real, t_lo) in enumerate(_GROUPS):
                for m, (m0, mm) in enumerate(KCH):
                    nc.scalar.activation(xa[gi][m][:mm, :ng],
                                         hsb[gi][m][:mm, :ng], Lrelu,
                                         bias=sft[m][:mm, :1],
                                         scale=scl[m][:mm, :1],
                                         alpha=LRELU_ALPHA)
                if l == 0:
                    nm_emit(1, gi, g0, ng, xa[gi])
            if l == 0:
                if profile:
                    nc.sync.dma_start(out=ago[1][0:NPAD, :], in_=agi[1][:, :])
                else:
                    nc.gpsimd.collective_compute(
                        "AllGather", mybir.AluOpType.bypass, replica_groups=RG,
                        ins=[agi[1].ap().opt()], outs=[ago[1].ap().opt()])

        # =========================================================
        # MLP head
        # =========================================================
        for gi, (g0, ng, real, t_lo) in enumerate(_GROUPS):
            if real == 0:
                continue
            z1 = [sba.tile([mm, NG], f16, name=f"z1_{m}", tag=f"z1_{m}", bufs=2)
                  for m, (m0, mm) in enumerate(KCH)]
            for m, (m0, mm) in enumerate(KCH):
                pb = psB.tile([P, NG], f32, tag="pb")
                for k, (k0, kk) in enumerate(KCH):
                    nc.tensor.matmul(pb[:mm, :ng], w_o1_sb[k][:kk, m0:m0+mm],
                                     xa[gi][k][:kk, :ng], start=(k == 0), stop=(k == 2))
                nc.scalar.activation(z1[m][:mm, :ng], pb[:mm, :ng], Lrelu,
                                     bias=b_o1_sb[m][:, :1], alpha=LRELU_ALPHA)
            z2 = [sba.tile([mm, NG], f16, name=f"z2_{m}", tag=f"z2_{m}", bufs=2)
                  for m, (m0, mm) in enumerate(KCH)]
            for m, (m0, mm) in enumerate(KCH):
                pb = psB.tile([P, NG], f32, tag="pb")
                for k, (k0, kk) in enumerate(KCH):
                    nc.tensor.matmul(pb[:mm, :ng], w_o2_sb[k][:kk, m0:m0+mm],
                                     z1[k][:kk, :ng], start=(k == 0), stop=(k == 2))
                nc.scalar.activation(z2[m][:mm, :ng], pb[:mm, :ng], Lrelu,
                                     bias=b_o2_sb[m][:, :1], alpha=LRELU_ALPHA)
            po = psB.tile([P, NG], f32, tag="pb")
            for k, (k0, kk) in enumerate(KCH):
                nc.tensor.matmul(po[:2, :ng], w_o3_sb[k][:kk, :],
                                 z2[k][:kk, :ng], start=(k == 0), stop=(k == 2))
            osb = sba.tile([2, NG], f32, tag="osb", bufs=2)
            nc.scalar.activation(osb[:2, :ng], po[:2, :ng], Ident,
                                 bias=b_o3_sb[:, :1])
            nc.sync.dma_start(out=out[g0:g0+real, :].transpose([1, 0]),
                              in_=osb[:2, :real])

        ctx.close()
    return nc


def _make_runner(nc, n_cores):
    """Compile once; return (prepare, run, unpack) over PJRT/axon shard_map."""
    import jax
    from jax.sharding import Mesh, PartitionSpec, NamedSharding
    from jax.experimental.shard_map import shard_map
    import concourse.mybir as mybir
    from concourse import bass2jax
    from concourse.bass2jax import _bass_exec_p, install_neuronx_cc_hook

    install_neuronx_cc_hook()
    partition_name = nc.partition_id_tensor.name if nc.partition_id_tensor else None

    in_names, out_names, out_avals, zero_outs = [], [], [], []
    for alloc in nc.m.functions[0].allocations:
        if not isinstance(alloc, mybir.MemoryLocationSet):
            continue
        name = alloc.memorylocations[0].name
        if alloc.kind == "ExternalInput":
            if name != partition_name:
                in_names.append(name)
        elif alloc.kind == "ExternalOutput":
            shape = tuple(alloc.tensor_shape)
            dtype = mybir.dt.np(alloc.dtype)
            out_names.append(name)
            out_avals.append(jax.core.ShapedArray(shape, dtype))
            zero_outs.append(np.zeros(shape, dtype))
    n_params = len(in_names)
    n_outs = len(out_avals)
    all_in_names = list(in_names) + list(out_names)
    if partition_name is not None:
        all_in_names.append(partition_name)

    def _body(*args):
        operands = list(args)
        if partition_name is not None:
            operands.append(bass2jax.partition_id_tensor())
        outs = _bass_exec_p.bind(
            *operands,
            out_avals=tuple(out_avals),
            in_names=tuple(all_in_names),
            out_names=tuple(out_names),
            lowering_input_output_aliases=(),
            sim_require_finite=True,
            sim_require_nnan=True,
            nc=nc,
        )
        return tuple(outs)

    devices = jax.devices()[:n_cores]
    mesh = Mesh(np.asarray(devices), ("core",))
    in_specs = (PartitionSpec("core"),) * (n_params + n_outs)
    out_specs = (PartitionSpec("core"),) * len(out_names)
    sharded = jax.jit(
        shard_map(_body, mesh=mesh, in_specs=in_specs, out_specs=out_specs,
                  check_rep=False),
        keep_unused=True,
    )

    def prepare(in_maps):
        sh = NamedSharding(mesh, PartitionSpec("core"))
        concat_in = [
            np.concatenate([np.asarray(in_maps[c][name]) for c in range(n_cores)],
                           axis=0)
            for name in in_names
        ]
        concat_zeros = [
            np.zeros((n_cores * z.shape[0], *z.shape[1:]), z.dtype)
            for z in zero_outs
        ]
        args = [jax.device_put(a, sh) for a in concat_in + concat_zeros]
        for a in args:
            a.block_until_ready()
        return args

    def run(args):
        return sharded(*args)

    def unpack(outs):
        return [
            {name: np.asarray(outs[i]).reshape(n_cores, *out_avals[i].shape)[c]
             for i, name in enumerate(out_names)}
            for c in range(n_cores)
        ]

    return prepare, run, unpack


_CACHE = {}
_LAST_ARGS = None


def kernel(**inputs):
    global _LAST_ARGS
    import jax
    inp = {k: np.asarray(v) for k, v in inputs.items()}

    g = _prep_graph(inp['edge_index'], inp['edge_type'])
    T = g['T']

    if "k" not in _CACHE:
        nc = _build_nc(T, g['tile_start'])
        nc.compile()
        _CACHE["k"] = (_make_runner(nc, NCORES), T)
    (prepare, run, unpack), T_built = _CACHE["k"]
    assert T_built == T, "edge distribution changed between calls"

    f16 = np.float16
    f32 = np.float32
    ncat = np.concatenate([inp['num_prop'], inp['cat_prop']], axis=1)
    ncat_T = _shard_T(ncat)
    tweet_T = _shard_T(inp['tweet_emb'])
    user_T = _shard_T(inp['user_emb'])
    uname_T = _shard_T(inp['user_name_emb'])

    w_ncat = np.zeros((8, 128), f16)
    w_ncat[0:5, 0:64] = inp['w_num'].astype(f16)
    w_ncat[5:8, 64:128] = inp['w_cat'].astype(f16)

    common = dict(
        w_ncat=w_ncat,
        w_tw=inp['w_tweet'], w_us=inp['w_user'], w_un=inp['w_uname'],
        w_in=inp['w_in'],
        w_rel1=inp['w_rel1'], w_root1=inp['w_root1'],
        w_rel2=inp['w_rel2'], w_root2=inp['w_root2'],
        w_o1=inp['w_o1'], w_o2=inp['w_o2'], w_o3=inp['w_o3'],
    )
    common = {k: np.ascontiguousarray(v, dtype=f16) for k, v in common.items()}
    common_f32 = dict(
        b_c0=np.concatenate([inp['b_num'], inp['b_cat']])[:, None],
        b_tw=inp['b_tweet'][:, None], b_us=inp['b_user'][:, None],
        b_c2=inp['b_uname'][:, None], b_in=inp['b_in'][:, None],
        gamma1=inp['gamma1'][:, None], beta1=inp['beta1'][:, None],
        gamma2=inp['gamma2'][:, None], beta2=inp['beta2'][:, None],
        b_o1=inp['b_o1'][:, None], b_o2=inp['b_o2'][:, None],
        b_o3=inp['b_o3'][:, None],
    )
    common.update({k: np.ascontiguousarray(v, dtype=f32)
                   for k, v in common_f32.items()})

    in_maps = []
    for c in range(NCORES):
        m = dict(common)
        m['ncat_T'] = ncat_T[c]
        m['tweet_T'] = tweet_T[c]
        m['user_T'] = user_T[c]
        m['uname_T'] = uname_T[c]
        m['s_matT'] = g['S_T'][c]
        m['tsrc'] = g['tsrc_cols'][c]
        in_maps.append(m)

    args = prepare(in_maps)
    _LAST_ARGS = args
    outs = run(args)
    jax.block_until_ready(outs)
    res = unpack(outs)
    full = np.concatenate([res[c]['out'] for c in range(NCORES)], axis=0)
    return full
